# revision 44
# baseline (speedup 1.0000x reference)
"""MoE SwiGLU feed-forward (top-2 of 8 experts) on 8 Trainium2 NeuronCores.

Expert-parallel with host-side routing (the gate is tiny: 134 MFLOP on the
host vs 283 GFLOP of expert FFN on the device) and pairwise load balance:
  host: exact fp32 gating -> top-2 ids + renormalized combine weights.
        Experts are paired heavy+light (sorted counts: 1st+8th, ...); the
        two cores of a pair each take one half of the hidden dim (H/2) of
        BOTH experts, so per-core work tracks the pair mean, not the max
        expert. Token lists are padded to shared caps (capA for the heavy
        slot, capB for the light slot), gathered and transposed into
        block-major bf16 blocks for contiguous per-partition DMA.
  core 2p+s: streams half-s weights of its pair's two experts (bf16,
        SBUF-resident) and both token blocks, then per <=512-token block
        runs the half-hidden SwiGLU FFN on the PE in bf16 (feature-major,
        no on-device transposes, no indirect DMA), scaling by the combine
        weight on PSUM eviction, writing a dense [capA+capB, D] fp32
        partial.
  host: out[idx_e] += y  from both halves (each token lands in exactly 2
        expert lists x 2 halves).
"""

import sys

for p in ("/opt/trn_rl_repo", "/root/.axon_site/_ro/trn_rl_repo"):
    if p not in sys.path:
        sys.path.insert(0, p)

import numpy as np
import ml_dtypes

import concourse.bass as bass
import concourse.mybir as mybir
import concourse.tile as tile
from concourse import bacc
from concourse.bass_utils import run_bass_kernel_spmd

P = 128
D = 1024          # model dim
H = 2816          # ffn hidden dim
HH = H // 2       # per-core hidden half
E = 8             # experts
T = 8192          # tokens
DC = D // P       # 8 contraction chunks
JC2 = HH // P     # 11 hidden chunks per half
BT = 512          # max tokens per FFN block

f32 = mybir.dt.float32
bf16 = mybir.dt.bfloat16

_CACHE: dict = {}
RUN_KWARGS: dict = {}   # test hook: extra kwargs for run_bass_kernel_spmd
LAST_RESULT = None      # test hook: BassKernelResults of the last run


def _blocks(cap: int):
    # avoid a 1-tile tail block: N=128 matmuls are LDWEIGHTS-bound
    # (~107ns for 53ns of streaming), so split a remainder of 5 tiles
    # as 3+2 instead of 4+1
    blocks = []
    c0 = 0
    while c0 < cap:
        r = (cap - c0) // P
        nt = min(4 if r != 5 else 3, r)
        blocks.append((c0, nt * P))
        c0 += nt * P
    return blocks


def _build(capA: int, capB: int):
    tilesA, tilesB = capA // P, capB // P
    tiles = tilesA + tilesB
    cap = capA + capB
    nc = bacc.Bacc(None, target_bir_lowering=False, name="moe_pair")

    xg = nc.dram_tensor("xg", [P, DC * cap], bf16, kind="ExternalInput")
    # wg/wu host-packed jc-major: [p, ((jc*DC)+dc)*P + j] = w[jc*P+j, dc*P+p]
    wgaA = nc.dram_tensor("wgaA", [P, JC2 * DC * P], bf16, kind="ExternalInput")
    wuaA = nc.dram_tensor("wuaA", [P, JC2 * DC * P], bf16, kind="ExternalInput")
    wgaB = nc.dram_tensor("wgaB", [P, JC2 * DC * P], bf16, kind="ExternalInput")
    wuaB = nc.dram_tensor("wuaB", [P, JC2 * DC * P], bf16, kind="ExternalInput")
    wdTA = nc.dram_tensor("wdTA", [HH, D], bf16, kind="ExternalInput")
    wdTB = nc.dram_tensor("wdTB", [HH, D], bf16, kind="ExternalInput")
    gat_d = nc.dram_tensor("gat", [P, tiles], f32, kind="ExternalInput")
    y = nc.dram_tensor("y", [cap, D], f32, kind="ExternalOutput")

    with tile.TileContext(nc) as tc:
        with (
            tc.tile_pool(name="keep", bufs=1) as keep,
            tc.tile_pool(name="xv", bufs=2) as xvp,
            tc.tile_pool(name="hts", bufs=1) as htsp,
            tc.tile_pool(name="sg", bufs=2) as sgp,
            tc.tile_pool(name="ysb", bufs=2) as ysbp,
            tc.tile_pool(name="wps", bufs=1, space="PSUM") as wpsp,
            tc.tile_pool(name="pgu", bufs=4, space="PSUM") as pgup,
            tc.tile_pool(name="pyp", bufs=3, space="PSUM") as pyp,
        ):
            junk = keep.tile([P, 2 * P], bf16, name="junk")
            nc.gpsimd.memset(junk[:], 0.0)
            gat = keep.tile([P, tiles], f32, name="gat")
            nc.gpsimd.dma_start(gat[:], gat_d.ap())

            # combined block list: (xg col offset, tb, part, tile offset)
            blocks = [(c0, tb, 0, c0 // P) for c0, tb in _blocks(capA)]
            blocks += [(capA + c0, tb, 1, tilesA + c0 // P)
                       for c0, tb in _blocks(capB)]

            def load_block(bi, eng):
                c0, tb, _, _ = blocks[bi]
                xv = xvp.tile([P, DC, tb], bf16, name="xv")
                eng.dma_start(
                    xv[:], xg.ap()[:, c0 * DC:(c0 + tb) * DC]
                    .rearrange("p (dc t) -> p dc t", dc=DC))
                return xv

            xv = load_block(0, nc.sync)

            # wg/wu stream in chunks on the two HWDGE rings right behind
            # the first two token blocks; the first block's matmuls
            # depend only on chunk 0 (~0.8MB). Expert B's weights and
            # both wd halves follow - they are needed much later.
            RW = DC * P

            def chunk_weights(src, eng, sizes, nm):
                tiles_, o = [], 0
                for k, njc in enumerate(sizes):
                    t = keep.tile([P, njc * RW], bf16, name=f"{nm}{k}")
                    eng.dma_start(t[:], src.ap()[:, o * RW:(o + njc) * RW])
                    tiles_.append((o, t))
                    o += njc
                return tiles_

            def wslice(chunks, jc, dc):
                for o, t in reversed(chunks):
                    if jc >= o:
                        return t[:, (jc - o) * RW + dc * P:(jc - o) * RW + (dc + 1) * P]
                raise AssertionError

            # ramp: xv0 rides sync alone while wg0+wu0 lead the scalar
            # ring, so the first matmul waits only on the larger of the
            # two rings (~13us) instead of their sum
            wg_chA = chunk_weights(wgaA, nc.scalar, [3], "wgA0c")
            wu_chA = chunk_weights(wuaA, nc.scalar, [3], "wuA0c")
            xv_next = load_block(1, nc.scalar)
            o = 3
            for k, njc in enumerate([4, 4]):
                t = keep.tile([P, njc * RW], bf16, name=f"wgA{k + 1}")
                nc.sync.dma_start(t[:], wgaA.ap()[:, o * RW:(o + njc) * RW])
                wg_chA.append((o, t))
                t = keep.tile([P, njc * RW], bf16, name=f"wuA{k + 1}")
                nc.scalar.dma_start(t[:], wuaA.ap()[:, o * RW:(o + njc) * RW])
                wu_chA.append((o, t))
                o += njc
            wdsA = keep.tile([P, JC2, D], bf16, name="wdsA")
            nc.sync.dma_start(wdsA[:], wdTA.ap().rearrange("(jc p) d -> p jc d", p=P))
            wg_chB = chunk_weights(wgaB, nc.sync, [11], "wgB")
            wu_chB = chunk_weights(wuaB, nc.scalar, [11], "wuB")
            wdsB = keep.tile([P, JC2, D], bf16, name="wdsB")
            nc.scalar.dma_start(wdsB[:], wdTB.ap().rearrange("(jc p) d -> p jc d", p=P))

            # keep the PE busy (HAM warm-up) until chunk 0 lands; the
            # junk tile is uninitialized SBUF - the product is never read
            wps = wpsp.tile([P, P], f32, name="wps")
            for _ in range(116):
                nc.tensor.matmul(wps[:], junk[:, :P], junk[:, P:],
                                 start=True, stop=True)

            for bi, (c0, tb, part, tile0) in enumerate(blocks):
                nt = tb // P
                wg_ch = wg_chA if part == 0 else wg_chB
                wu_ch = wu_chA if part == 0 else wu_chB
                wds = wdsA if part == 0 else wdsB
                # h = silu(x @ wg) * (x @ wu), one 128-chunk of hidden at a time
                hts = htsp.tile([P, JC2, tb], bf16, name="hts")
                for jc in range(JC2):
                    pg = pgup.tile([P, tb], f32, name="pg", tag="gu")
                    pu = pgup.tile([P, tb], f32, name="pu", tag="gu")
                    for dc in range(DC):
                        nc.tensor.matmul(
                            pg[:], wslice(wg_ch, jc, dc), xv[:, dc, :],
                            start=(dc == 0), stop=(dc == DC - 1),
                        )
                    for dc in range(DC):
                        nc.tensor.matmul(
                            pu[:], wslice(wu_ch, jc, dc), xv[:, dc, :],
                            start=(dc == 0), stop=(dc == DC - 1),
                        )
                    sg = sgp.tile([P, tb], f32, name="sg")
                    nc.scalar.activation(sg[:], pg[:], mybir.ActivationFunctionType.Silu)
                    nc.vector.tensor_mul(hts[:, jc, :], sg[:], pu[:])
                # prefetch the next block's tokens behind this block's matmuls
                xv = xv_next
                if bi + 2 < len(blocks):
                    xv_next = load_block(bi + 2,
                                         nc.sync if bi % 2 == 0 else nc.scalar)
                # y = (h @ wd) * combine_weight, per 128-token tile
                for tt in range(nt):
                    g = tile0 + tt
                    ysb = ysbp.tile([P, D], f32, name="ysb")
                    for ddh in range(2):
                        py = pyp.tile([P, 512], f32, name="py")
                        for jc in range(JC2):
                            nc.tensor.matmul(
                                py[:], hts[:, jc, tt * P:(tt + 1) * P],
                                wds[:, jc, ddh * 512:(ddh + 1) * 512],
                                start=(jc == 0), stop=(jc == JC2 - 1),
                            )
                        nc.scalar.activation(
                            ysb[:, ddh * 512:(ddh + 1) * 512], py[:],
                            mybir.ActivationFunctionType.Copy,
                            scale=gat[:, g:g + 1],
                        )
                    nc.sync.dma_start(y.ap()[g * P:(g + 1) * P, :], ysb[:])

    nc.compile()
    return nc


def kernel(x, gate_w, wg, wu, wd):
    xf = np.ascontiguousarray(np.asarray(x, dtype=np.float32).reshape(T, D))
    gw = np.asarray(gate_w, dtype=np.float32)
    wg = np.asarray(wg, dtype=np.float32)
    wu = np.asarray(wu, dtype=np.float32)
    wd = np.asarray(wd, dtype=np.float32)

    # exact fp32 routing on the host
    logits = xf @ gw.T
    m = logits.max(axis=1, keepdims=True)
    sc = np.exp(logits - m)
    sc /= sc.sum(axis=1, keepdims=True)
    top2 = np.argpartition(-sc, 2, axis=1)[:, :2]
    tw = np.take_along_axis(sc, top2, axis=1)
    tw = tw / tw.sum(axis=1, keepdims=True)

    idxs, wts = [], []
    for e in range(E):
        sel = (top2 == e)
        rows = np.where(sel.any(axis=1))[0]
        w = (tw * sel)[rows].sum(axis=1)
        idxs.append(rows)
        wts.append(w.astype(np.float32))
    cnts = np.array([len(r) for r in idxs])

    # pair heavy with light: sorted desc, 1st+8th, 2nd+7th, ...
    order = np.argsort(-cnts, kind="stable")
    pairs = [(int(order[i]), int(order[E - 1 - i])) for i in range(E // 2)]
    capA = max(128, -(-max(cnts[a] for a, _ in pairs) // P) * P)
    capB = max(128, -(-max(cnts[b] for _, b in pairs) // P) * P)
    if (capA, capB) not in _CACHE:
        _CACHE[(capA, capB)] = _build(capA, capB)
    nc = _CACHE[(capA, capB)]
    tilesA, tilesB = capA // P, capB // P

    xbf = xf.astype(ml_dtypes.bfloat16)

    def pack_x(idx, cap):
        # gathered+transposed token block: xgT[d, t] = x[idx[t], d],
        # block-major: block (c0, tb) occupies columns [c0*DC, (c0+tb)*DC)
        # as [dc, t] (contiguous per block)
        xgT = np.zeros((D, cap), dtype=ml_dtypes.bfloat16)
        xgT[:, :len(idx)] = xbf[idx].T
        return [xgT[:, c0:c0 + tb].reshape(DC, P, tb)
                .transpose(1, 0, 2).reshape(P, DC * tb)
                for c0, tb in _blocks(cap)]

    def pack_jc(w):
        # [HH, D] -> [P, JC2*DC*P] with [p, ((jc*DC)+dc)*P + j] = w[jc*P+j, dc*P+p]
        return np.ascontiguousarray(
            w.reshape(JC2, P, DC, P).transpose(3, 0, 2, 1).reshape(P, JC2 * DC * P)
        ).astype(ml_dtypes.bfloat16)

    in_maps = [None] * E
    for pi, (a, b) in enumerate(pairs):
        xgn = np.ascontiguousarray(np.concatenate(
            pack_x(idxs[a], capA) + pack_x(idxs[b], capB), axis=1))
        gflat = np.zeros(capA + capB, dtype=np.float32)
        gflat[:cnts[a]] = wts[a]
        gflat[capA:capA + cnts[b]] = wts[b]
        gatn = np.ascontiguousarray(
            gflat.reshape(tilesA + tilesB, P).T)   # slot g*128+p <-> (p, g)
        for s in range(2):
            hs = slice(s * HH, (s + 1) * HH)
            in_maps[2 * pi + s] = {
                "xg": xgn,
                "gat": gatn,
                "wgaA": pack_jc(wg[a][hs]),
                "wuaA": pack_jc(wu[a][hs]),
                "wgaB": pack_jc(wg[b][hs]),
                "wuaB": pack_jc(wu[b][hs]),
                "wdTA": np.ascontiguousarray(wd[a].T[hs]).astype(ml_dtypes.bfloat16),
                "wdTB": np.ascontiguousarray(wd[b].T[hs]).astype(ml_dtypes.bfloat16),
            }
    res = run_bass_kernel_spmd(nc, in_maps, core_ids=list(range(E)), **RUN_KWARGS)
    globals()["LAST_RESULT"] = res
    out = np.zeros((T, D), dtype=np.float32)
    for pi, (a, b) in enumerate(pairs):
        for s in range(2):
            ye = res.results[2 * pi + s]["y"]
            out[idxs[a]] += ye[:cnts[a]]
            out[idxs[b]] += ye[capA:capA + cnts[b]]
    return out.reshape(np.asarray(x).shape)


# revision 45
# speedup vs baseline: 1.0012x; 1.0012x over previous
"""MoE SwiGLU feed-forward (top-2 of 8 experts) on 8 Trainium2 NeuronCores.

Expert-parallel with host-side routing (the gate is tiny: 134 MFLOP on the
host vs 283 GFLOP of expert FFN on the device) and pairwise load balance:
  host: exact fp32 gating -> top-2 ids + renormalized combine weights.
        Experts are paired heavy+light (sorted counts: 1st+8th, ...); the
        two cores of a pair each take one half of the hidden dim (H/2) of
        BOTH experts, so per-core work tracks the pair mean, not the max
        expert. Token lists are padded to shared caps (capA for the heavy
        slot, capB for the light slot), gathered and transposed into
        block-major bf16 blocks for contiguous per-partition DMA.
  core 2p+s: streams half-s weights of its pair's two experts (bf16,
        SBUF-resident) and both token blocks, then per <=512-token block
        runs the half-hidden SwiGLU FFN on the PE in bf16 (feature-major,
        no on-device transposes, no indirect DMA), scaling by the combine
        weight on PSUM eviction, writing a dense [capA+capB, D] fp32
        partial.
  host: out[idx_e] += y  from both halves (each token lands in exactly 2
        expert lists x 2 halves).
"""

import sys

for p in ("/opt/trn_rl_repo", "/root/.axon_site/_ro/trn_rl_repo"):
    if p not in sys.path:
        sys.path.insert(0, p)

import numpy as np
import ml_dtypes

import concourse.bass as bass
import concourse.mybir as mybir
import concourse.tile as tile
from concourse import bacc
from concourse.bass_utils import run_bass_kernel_spmd

P = 128
D = 1024          # model dim
H = 2816          # ffn hidden dim
HH = H // 2       # per-core hidden half
E = 8             # experts
T = 8192          # tokens
DC = D // P       # 8 contraction chunks
JC2 = HH // P     # 11 hidden chunks per half
BT = 512          # max tokens per FFN block

f32 = mybir.dt.float32
bf16 = mybir.dt.bfloat16

_CACHE: dict = {}
RUN_KWARGS: dict = {}   # test hook: extra kwargs for run_bass_kernel_spmd
LAST_RESULT = None      # test hook: BassKernelResults of the last run


def _blocks(cap: int):
    # avoid a 1-tile tail block: N=128 matmuls are LDWEIGHTS-bound
    # (~107ns for 53ns of streaming), so split a remainder of 5 tiles
    # as 3+2 instead of 4+1
    blocks = []
    c0 = 0
    while c0 < cap:
        r = (cap - c0) // P
        nt = min(4 if r != 5 else 3, r)
        blocks.append((c0, nt * P))
        c0 += nt * P
    return blocks


def _build(capA: int, capB: int):
    tilesA, tilesB = capA // P, capB // P
    tiles = tilesA + tilesB
    cap = capA + capB
    nc = bacc.Bacc(None, target_bir_lowering=False, name="moe_pair")

    xg = nc.dram_tensor("xg", [P, DC * cap], bf16, kind="ExternalInput")
    # wg/wu host-packed jc-major: [p, ((jc*DC)+dc)*P + j] = w[jc*P+j, dc*P+p]
    wgaA = nc.dram_tensor("wgaA", [P, JC2 * DC * P], bf16, kind="ExternalInput")
    wuaA = nc.dram_tensor("wuaA", [P, JC2 * DC * P], bf16, kind="ExternalInput")
    wgaB = nc.dram_tensor("wgaB", [P, JC2 * DC * P], bf16, kind="ExternalInput")
    wuaB = nc.dram_tensor("wuaB", [P, JC2 * DC * P], bf16, kind="ExternalInput")
    wdTA = nc.dram_tensor("wdTA", [HH, D], bf16, kind="ExternalInput")
    wdTB = nc.dram_tensor("wdTB", [HH, D], bf16, kind="ExternalInput")
    gat_d = nc.dram_tensor("gat", [P, tiles], f32, kind="ExternalInput")
    y = nc.dram_tensor("y", [cap, D], f32, kind="ExternalOutput")

    with tile.TileContext(nc) as tc:
        with (
            tc.tile_pool(name="keep", bufs=1) as keep,
            tc.tile_pool(name="xv", bufs=2) as xvp,
            tc.tile_pool(name="hts", bufs=1) as htsp,
            tc.tile_pool(name="sg", bufs=2) as sgp,
            tc.tile_pool(name="ysb", bufs=2) as ysbp,
            tc.tile_pool(name="wps", bufs=1, space="PSUM") as wpsp,
            tc.tile_pool(name="pgu", bufs=4, space="PSUM") as pgup,
            tc.tile_pool(name="pyp", bufs=3, space="PSUM") as pyp,
        ):
            junk = keep.tile([P, 2 * P], bf16, name="junk")
            nc.gpsimd.memset(junk[:], 0.0)
            gat = keep.tile([P, tiles], f32, name="gat")
            nc.gpsimd.dma_start(gat[:], gat_d.ap())

            # combined block list: (xg col offset, tb, part, tile offset)
            blocks = [(c0, tb, 0, c0 // P) for c0, tb in _blocks(capA)]
            blocks += [(capA + c0, tb, 1, tilesA + c0 // P)
                       for c0, tb in _blocks(capB)]

            def load_block(bi, eng):
                c0, tb, _, _ = blocks[bi]
                xv = xvp.tile([P, DC, tb], bf16, name="xv")
                eng.dma_start(
                    xv[:], xg.ap()[:, c0 * DC:(c0 + tb) * DC]
                    .rearrange("p (dc t) -> p dc t", dc=DC))
                return xv

            xv = load_block(0, nc.sync)

            # wg/wu stream in chunks on the two HWDGE rings right behind
            # the first two token blocks; the first block's matmuls
            # depend only on chunk 0 (~0.8MB). Expert B's weights and
            # both wd halves follow - they are needed much later.
            RW = DC * P

            def chunk_weights(src, eng, sizes, nm):
                tiles_, o = [], 0
                for k, njc in enumerate(sizes):
                    t = keep.tile([P, njc * RW], bf16, name=f"{nm}{k}")
                    eng.dma_start(t[:], src.ap()[:, o * RW:(o + njc) * RW])
                    tiles_.append((o, t))
                    o += njc
                return tiles_

            def wslice(chunks, jc, dc):
                for o, t in reversed(chunks):
                    if jc >= o:
                        return t[:, (jc - o) * RW + dc * P:(jc - o) * RW + (dc + 1) * P]
                raise AssertionError

            xv_next = load_block(1, nc.scalar)
            wg_chA = chunk_weights(wgaA, nc.sync, [3, 4, 4], "wgA")
            wu_chA = chunk_weights(wuaA, nc.scalar, [3, 4, 4], "wuA")
            wdsA = keep.tile([P, JC2, D], bf16, name="wdsA")
            nc.sync.dma_start(wdsA[:], wdTA.ap().rearrange("(jc p) d -> p jc d", p=P))
            wg_chB = chunk_weights(wgaB, nc.sync, [11], "wgB")
            wu_chB = chunk_weights(wuaB, nc.scalar, [11], "wuB")
            wdsB = keep.tile([P, JC2, D], bf16, name="wdsB")
            nc.scalar.dma_start(wdsB[:], wdTB.ap().rearrange("(jc p) d -> p jc d", p=P))

            # keep the PE busy (HAM warm-up) until chunk 0 lands; the
            # junk tile is uninitialized SBUF - the product is never read
            wps = wpsp.tile([P, P], f32, name="wps")
            for _ in range(116):
                nc.tensor.matmul(wps[:], junk[:, :P], junk[:, P:],
                                 start=True, stop=True)

            for bi, (c0, tb, part, tile0) in enumerate(blocks):
                nt = tb // P
                wg_ch = wg_chA if part == 0 else wg_chB
                wu_ch = wu_chA if part == 0 else wu_chB
                wds = wdsA if part == 0 else wdsB
                # h = silu(x @ wg) * (x @ wu), one 128-chunk of hidden at a time
                hts = htsp.tile([P, JC2, tb], bf16, name="hts")
                for jc in range(JC2):
                    pg = pgup.tile([P, tb], f32, name="pg", tag="gu")
                    pu = pgup.tile([P, tb], f32, name="pu", tag="gu")
                    for dc in range(DC):
                        nc.tensor.matmul(
                            pg[:], wslice(wg_ch, jc, dc), xv[:, dc, :],
                            start=(dc == 0), stop=(dc == DC - 1),
                        )
                    for dc in range(DC):
                        nc.tensor.matmul(
                            pu[:], wslice(wu_ch, jc, dc), xv[:, dc, :],
                            start=(dc == 0), stop=(dc == DC - 1),
                        )
                    sg = sgp.tile([P, tb], f32, name="sg")
                    nc.scalar.activation(sg[:], pg[:], mybir.ActivationFunctionType.Silu)
                    nc.vector.tensor_mul(hts[:, jc, :], sg[:], pu[:])
                # prefetch the next block's tokens behind this block's matmuls
                xv = xv_next
                if bi + 2 < len(blocks):
                    xv_next = load_block(bi + 2,
                                         nc.sync if bi % 2 == 0 else nc.scalar)
                # y = (h @ wd) * combine_weight, per 128-token tile
                for tt in range(nt):
                    g = tile0 + tt
                    ysb = ysbp.tile([P, D], f32, name="ysb")
                    for ddh in range(2):
                        py = pyp.tile([P, 512], f32, name="py")
                        for jc in range(JC2):
                            nc.tensor.matmul(
                                py[:], hts[:, jc, tt * P:(tt + 1) * P],
                                wds[:, jc, ddh * 512:(ddh + 1) * 512],
                                start=(jc == 0), stop=(jc == JC2 - 1),
                            )
                        nc.scalar.activation(
                            ysb[:, ddh * 512:(ddh + 1) * 512], py[:],
                            mybir.ActivationFunctionType.Copy,
                            scale=gat[:, g:g + 1],
                        )
                    nc.sync.dma_start(y.ap()[g * P:(g + 1) * P, :], ysb[:])

    nc.compile()
    return nc


def kernel(x, gate_w, wg, wu, wd):
    xf = np.ascontiguousarray(np.asarray(x, dtype=np.float32).reshape(T, D))
    gw = np.asarray(gate_w, dtype=np.float32)
    wg = np.asarray(wg, dtype=np.float32)
    wu = np.asarray(wu, dtype=np.float32)
    wd = np.asarray(wd, dtype=np.float32)

    # exact fp32 routing on the host
    logits = xf @ gw.T
    m = logits.max(axis=1, keepdims=True)
    sc = np.exp(logits - m)
    sc /= sc.sum(axis=1, keepdims=True)
    top2 = np.argpartition(-sc, 2, axis=1)[:, :2]
    tw = np.take_along_axis(sc, top2, axis=1)
    tw = tw / tw.sum(axis=1, keepdims=True)

    idxs, wts = [], []
    for e in range(E):
        sel = (top2 == e)
        rows = np.where(sel.any(axis=1))[0]
        w = (tw * sel)[rows].sum(axis=1)
        idxs.append(rows)
        wts.append(w.astype(np.float32))
    cnts = np.array([len(r) for r in idxs])

    # pair heavy with light: sorted desc, 1st+8th, 2nd+7th, ...
    order = np.argsort(-cnts, kind="stable")
    pairs = [(int(order[i]), int(order[E - 1 - i])) for i in range(E // 2)]
    capA = max(128, -(-max(cnts[a] for a, _ in pairs) // P) * P)
    capB = max(128, -(-max(cnts[b] for _, b in pairs) // P) * P)
    if (capA, capB) not in _CACHE:
        _CACHE[(capA, capB)] = _build(capA, capB)
    nc = _CACHE[(capA, capB)]
    tilesA, tilesB = capA // P, capB // P

    xbf = xf.astype(ml_dtypes.bfloat16)

    def pack_x(idx, cap):
        # gathered+transposed token block: xgT[d, t] = x[idx[t], d],
        # block-major: block (c0, tb) occupies columns [c0*DC, (c0+tb)*DC)
        # as [dc, t] (contiguous per block)
        xgT = np.zeros((D, cap), dtype=ml_dtypes.bfloat16)
        xgT[:, :len(idx)] = xbf[idx].T
        return [xgT[:, c0:c0 + tb].reshape(DC, P, tb)
                .transpose(1, 0, 2).reshape(P, DC * tb)
                for c0, tb in _blocks(cap)]

    def pack_jc(w):
        # [HH, D] -> [P, JC2*DC*P] with [p, ((jc*DC)+dc)*P + j] = w[jc*P+j, dc*P+p]
        return np.ascontiguousarray(
            w.reshape(JC2, P, DC, P).transpose(3, 0, 2, 1).reshape(P, JC2 * DC * P)
        ).astype(ml_dtypes.bfloat16)

    in_maps = [None] * E
    for pi, (a, b) in enumerate(pairs):
        xgn = np.ascontiguousarray(np.concatenate(
            pack_x(idxs[a], capA) + pack_x(idxs[b], capB), axis=1))
        gflat = np.zeros(capA + capB, dtype=np.float32)
        gflat[:cnts[a]] = wts[a]
        gflat[capA:capA + cnts[b]] = wts[b]
        gatn = np.ascontiguousarray(
            gflat.reshape(tilesA + tilesB, P).T)   # slot g*128+p <-> (p, g)
        for s in range(2):
            hs = slice(s * HH, (s + 1) * HH)
            in_maps[2 * pi + s] = {
                "xg": xgn,
                "gat": gatn,
                "wgaA": pack_jc(wg[a][hs]),
                "wuaA": pack_jc(wu[a][hs]),
                "wgaB": pack_jc(wg[b][hs]),
                "wuaB": pack_jc(wu[b][hs]),
                "wdTA": np.ascontiguousarray(wd[a].T[hs]).astype(ml_dtypes.bfloat16),
                "wdTB": np.ascontiguousarray(wd[b].T[hs]).astype(ml_dtypes.bfloat16),
            }
    res = run_bass_kernel_spmd(nc, in_maps, core_ids=list(range(E)), **RUN_KWARGS)
    globals()["LAST_RESULT"] = res
    out = np.zeros((T, D), dtype=np.float32)
    for pi, (a, b) in enumerate(pairs):
        for s in range(2):
            ye = res.results[2 * pi + s]["y"]
            out[idxs[a]] += ye[:cnts[a]]
            out[idxs[b]] += ye[capA:capA + cnts[b]]
    return out.reshape(np.asarray(x).shape)
